# revision 6
# baseline (speedup 1.0000x reference)
"""Trainium2 Bass kernel for a 2-layer LIF spiking network scan.

Reference computation (per timestep t, beta=0.95, threshold=1.0):
    cur1 = x_t @ w1.T + b1
    mem1 = beta*mem1 + cur1 - H(mem1_prev - 1)     (reset-by-subtract)
    spk1 = H(mem1 - 1)
    cur2 = spk1 @ w2.T + b2
    mem2 = beta*mem2 + cur2 - H(mem2_prev - 1)
    spk2 = H(mem2 - 1)                              -> output [T, B, 10]

Sharding: data-parallel over batch (512 -> 8 cores x 64), weights replicated,
temporal scan local per core.

Numerics: the scan is chaotic (output spikes are bit-sensitive to ~1e-6
perturbations of cur1), so matmuls must be fp32-exact. The PE's native fp32
mode runs at 1/4 rate, so layer-1 instead uses an exact fp16 hi/lo split:
x = xh + xl, w = wh + wl (fp16 each, products exact in fp32 PSUM), keeping
xh*wh + xh*wl + xl*wh (dropped xl*wl term ~2^-22). The three passes are
K-packed into one 2432-row contraction [xh;xl;xh] x [wl;wh;wh] (pad to
19 k-tiles of 128) running at full bf16-class PE rate - 3x faster than
native fp32 with end-to-end spike flips indistinguishable from fp32.
Correction rows accumulate into PSUM before the hi*hi rows so their
rounding happens at small magnitudes.

Per-core design (B_l=64; hidden 1024 = 8 chunks x 128 partitions, free dim
= (chunk, batch) = 512):
  - Host pre-transposes x to [128, 19, T*64] fp16 so the contraction lands
    on partitions with fully contiguous DMA (2KB runs).
  - Per time-chunk of TC=8 steps: 152 matmuls (N=512) accumulate cur1 into
    PSUM, rotating over 7 banks; ScalarE evacuates PSUM->SBUF adding b1 via
    the per-partition activation bias.
  - The sequential LIF scan runs on VectorE with fused scalar_tensor_tensor
    ops: mem = (mem*0.95)+cur ; mem = (spk_prev*-1)+mem ; spk = (mem > 1).
    Spikes are exact {0.0, 1.0} stored as fp16 (feeds layer 2 directly).
  - Layer 2 lags one chunk: w2's hi/lo fp16 parts are stacked column-wise
    [w2h | pad | w2l] (42 cols, 32-aligned) so ONE matmul per k-tile (8 of
    N=512, PSUM bank 8) computes both halves concurrently on disjoint
    output partitions; ScalarE+VectorE sum the halves with b2, then the
    same scan pattern on [10, 64] tiles, and spk2 streams out per chunk.

Engine budget per core (TimelineSim, validated on HW within ~10%):
PE ~859us busy of ~903us total; DVE ~390us; ACT ~139us; DMA ~190us.
"""

import numpy as np
from contextlib import ExitStack

import concourse.bass as bass
import concourse.mybir as mybir
import concourse.tile as tile
from concourse import bacc
from concourse import bass_utils
from concourse.alu_op_type import AluOpType

F32 = mybir.dt.float32
AF = mybir.ActivationFunctionType

T, B, NIN, NHID, NOUT = 200, 512, 784, 1024, 10
NCORES = 8
BL = B // NCORES            # 64 batch per core
KT, KP = 7, 112             # input k-tiles
MT = 8                      # hidden chunks of 128
TC = 8                      # timesteps per chunk
CB = TC * BL                # 512 columns per chunk
BETA = 0.95
THR = 1.0


KR = 19            # packed k-tiles of 128 for fp16pack: [xh; xh; xl] + pad


def build_nc(t_total=T, mode="fp32", repeat=1):
    """mode: fp32 | fp16split | fp16pack | f32r; repeat: timing-only loops"""
    nch = t_total // TC
    F16 = mybir.dt.float16
    F32R = mybir.dt.float32r
    nc = bacc.Bacc("TRN2", target_bir_lowering=False, debug=False,
                   num_devices=NCORES)
    split = mode in ("fp16split", "fp16pack")
    packed = mode == "fp16pack"
    # f32r: tiles/DMA stay plain fp32 (walrus rejects memset/copy on f32r);
    # the PE mode flag is applied via .bitcast(F32R) on matmul operands only.
    f32r = mode == "f32r"
    mm_dt = {"fp32": F32, "fp16split": F16, "fp16pack": F16,
             "f32r": F32}[mode]
    s1_dt = F16 if split else mm_dt

    def mmcast(ap):
        return ap.bitcast(F32R) if f32r else ap

    xparts = ["xh", "xl"] if (split and not packed) else ["xh"]
    wparts = ["wh", "wl"] if split else ["wh"]
    # (xpart, wpart) term list for layer 1 / layer 2
    terms1 = [("xh", "wh"), ("xl", "wh"), ("xh", "wl")] \
        if (split and not packed) else [("xh", "wh")]
    terms2 = ["wh", "wl"] if split else ["wh"]

    if packed:
        x_d = {"xh": nc.dram_tensor("x_xh", [128, KR, t_total * BL], mm_dt,
                                    kind="ExternalInput")}
        w1_d = {"wh": nc.dram_tensor("w1_wh", [128, KR, NHID], mm_dt,
                                     kind="ExternalInput")}
    else:
        x_d = {p: nc.dram_tensor(f"x_{p}", [KP, KT, t_total * BL], mm_dt,
                                 kind="ExternalInput") for p in xparts}
        w1_d = {p: nc.dram_tensor(f"w1_{p}", [KP, KT, NHID], mm_dt,
                                  kind="ExternalInput") for p in wparts}
    b1_d = nc.dram_tensor("b1_in", [128, MT], F32, kind="ExternalInput")
    w2_d = {"wh": nc.dram_tensor("w2_wh",
                                 [128, MT, (32 + NOUT) if split else NOUT],
                                 mm_dt, kind="ExternalInput")}
    b2_d = nc.dram_tensor("b2_in", [NOUT, 1], F32, kind="ExternalInput")
    w2_ncol = (32 + NOUT) if split else NOUT
    o_d = nc.dram_tensor("o_out", [NOUT, t_total * BL], F32,
                         kind="ExternalOutput")

    with tile.TileContext(nc) as tc, ExitStack() as ctx:
        consts = ctx.enter_context(tc.tile_pool(name="consts", bufs=1))
        xpool = ctx.enter_context(tc.tile_pool(name="xpool", bufs=3))
        curp = ctx.enter_context(tc.tile_pool(name="curp", bufs=2))
        s1p = ctx.enter_context(tc.tile_pool(name="s1p", bufs=2))
        cur2p = ctx.enter_context(tc.tile_pool(name="cur2p", bufs=2))
        outp = ctx.enter_context(tc.tile_pool(name="outp", bufs=2))
        state = ctx.enter_context(tc.tile_pool(name="state", bufs=1))
        psl1 = ctx.enter_context(tc.tile_pool(name="psl1", bufs=7,
                                              space="PSUM"))
        psl2 = ctx.enter_context(tc.tile_pool(name="psl2", bufs=1,
                                              space="PSUM"))

        w1_sb = {}
        for p in w1_d:
            shp = [128, KR, NHID] if packed else [KP, KT, NHID]
            w1_sb[p] = consts.tile(shp, mm_dt, name=f"w1sb_{p}",
                                   tag=f"w1sb_{p}")
            for ki in range(shp[1]):
                nc.sync.dma_start(w1_sb[p][:, ki, :],
                                  w1_d[p].ap()[:, ki, :])
        b1_sb = consts.tile([128, MT], F32)
        nc.sync.dma_start(b1_sb[:], b1_d.ap()[:])
        w2_sb = {}
        for p in w2_d:
            ncol = (32 + NOUT) if (split and p == "wh") else NOUT
            w2_sb[p] = consts.tile([128, MT, ncol], mm_dt, name=f"w2sb_{p}",
                                   tag=f"w2sb_{p}")
            nc.sync.dma_start(w2_sb[p][:], w2_d[p].ap()[:])
        b2_sb = consts.tile([NOUT, 1], F32)
        nc.sync.dma_start(b2_sb[:], b2_d.ap()[:])

        mem1 = state.tile([128, MT * BL], F32)
        nc.vector.memset(mem1[:], 0.0)
        mem2 = state.tile([NOUT, BL], F32)
        nc.vector.memset(mem2[:], 0.0)
        s1_init = state.tile([128, MT * BL], s1_dt)
        nc.vector.memset(s1_init[:], 0.0)
        s2_init = state.tile([NOUT, BL], F32)
        nc.vector.memset(s2_init[:], 0.0)

        # pipeline-carried python refs
        cur_tiles = {}      # c -> cur1 SBUF tile [128, TC, 512]
        s1_tiles = {}       # c -> spike chunk [128, TC, 512]
        spk1_prev_slice = s1_init[:]
        spk2_prev_slice = s2_init[:]

        def dma_and_mm(c):
            x_c = {}
            for p in xparts:
                shp = [128, KR, CB] if packed else [KP, KT, CB]
                x_c[p] = xpool.tile(shp, mm_dt, name=f"x_c_{p}",
                                    tag=f"x_c_{p}")
                if c == 0:
                    for ki in range(shp[1]):
                        nc.sync.dma_start(
                            x_c[p][:, ki, :],
                            x_d[p].ap()[:, ki, c * CB:(c + 1) * CB])
                else:
                    nc.sync.dma_start(
                        x_c[p][:],
                        x_d[p].ap()[:, :, c * CB:(c + 1) * CB])
            cur_c = curp.tile([128, TC, MT * BL], F32, name="cur_c",
                              tag="cur_c")
            nterm = len(terms1)
            for mi in range(MT):
                ps = psl1.tile([128, CB], F32, name="ps_l1", tag="ps_l1")
                if packed:
                    for ki in range(KR):
                        nc.tensor.matmul(
                            ps[:],
                            w1_sb["wh"][:, ki, mi * 128:(mi + 1) * 128],
                            x_c["xh"][:, ki, :],
                            start=(ki == 0),
                            stop=(ki == KR - 1),
                        )
                else:
                    for ti, (xp, wp) in enumerate(terms1):
                        for ki in range(KT):
                            nc.tensor.matmul(
                                ps[:],
                                mmcast(w1_sb[wp][:, ki,
                                                 mi * 128:(mi + 1) * 128]),
                                mmcast(x_c[xp][:, ki, :]),
                                start=(ti == 0 and ki == 0),
                                stop=(ti == nterm - 1 and ki == KT - 1),
                            )
                nc.scalar.activation(
                    cur_c[:, :, mi * BL:(mi + 1) * BL],
                    ps.rearrange("p (t b) -> p t b", t=TC),
                    AF.Identity,
                    bias=b1_sb[:, mi:mi + 1],
                    scale=1.0,
                )
            cur_tiles[c] = cur_c

        def scan_l1(c):
            nonlocal spk1_prev_slice
            cur_c = cur_tiles.pop(c)
            s1_c = s1p.tile([128, TC, MT * BL], s1_dt, name="s1_c",
                            tag="s1_c")
            for t in range(TC):
                nc.vector.scalar_tensor_tensor(
                    mem1[:], mem1[:], BETA, cur_c[:, t, :],
                    op0=AluOpType.mult, op1=AluOpType.add)
                nc.vector.scalar_tensor_tensor(
                    mem1[:], spk1_prev_slice, -1.0, mem1[:],
                    op0=AluOpType.mult, op1=AluOpType.add)
                nc.vector.tensor_scalar(
                    s1_c[:, t, :], mem1[:], THR, None, AluOpType.is_gt)
                spk1_prev_slice = s1_c[:, t, :]
            s1_tiles[c] = s1_c

        def phase_l2(c):
            nonlocal spk2_prev_slice
            s1_c = s1_tiles.pop(c)
            ps2 = psl2.tile([w2_ncol, TC, BL], F32, name="ps2", tag="ps2")
            for ki in range(MT):
                nc.tensor.matmul(
                    ps2[:],
                    mmcast(w2_sb["wh"][:, ki, :]),
                    mmcast(s1_c[:, :, ki * BL:(ki + 1) * BL]),
                    start=(ki == 0),
                    stop=(ki == MT - 1),
                )
            cur2_c = cur2p.tile([NOUT, TC, BL], F32, name="cur2_c",
                                tag="cur2_c")
            if split:
                # halves: partitions 0-9 = x@w2h, 10-19 = x@w2l; sum + b2
                tmp2 = cur2p.tile([NOUT, TC, BL], F32, name="tmp2",
                                  tag="tmp2")
                nc.scalar.activation(tmp2[:], ps2[32:32 + NOUT, :, :],
                                     AF.Identity, bias=b2_sb[:], scale=1.0)
                nc.vector.scalar_tensor_tensor(
                    cur2_c[:], ps2[0:NOUT, :, :], 0.0, tmp2[:],
                    op0=AluOpType.bypass, op1=AluOpType.add)
            else:
                nc.scalar.activation(cur2_c[:], ps2[:], AF.Identity,
                                     bias=b2_sb[:], scale=1.0)
            out_c = outp.tile([NOUT, TC, BL], F32, name="out_c", tag="out_c")
            for t in range(TC):
                nc.vector.scalar_tensor_tensor(
                    mem2[:], mem2[:], BETA, cur2_c[:, t, :],
                    op0=AluOpType.mult, op1=AluOpType.add)
                nc.vector.scalar_tensor_tensor(
                    mem2[:], spk2_prev_slice, -1.0, mem2[:],
                    op0=AluOpType.mult, op1=AluOpType.add)
                nc.vector.tensor_scalar(
                    out_c[:, t, :], mem2[:], THR, None, AluOpType.is_gt)
                spk2_prev_slice = out_c[:, t, :]
            nc.sync.dma_start(
                o_d.ap()[:, c * CB:(c + 1) * CB],
                out_c.rearrange("p t b -> p (t b)"))

        for _r in range(repeat):
            for c in range(nch):
                dma_and_mm(c)
                if c >= 1:
                    scan_l1(c - 1)
                if c >= 2:
                    phase_l2(c - 2)
            scan_l1(nch - 1)
            if nch >= 2:
                phase_l2(nch - 2)
            phase_l2(nch - 1)

    nc.compile()
    return nc


def _prep_inputs(data, w1, b1, w2, b2, t_total=T, mode="fp32"):
    """Host-side sharding + layout transforms. Returns list of 8 in_maps."""
    split = mode in ("fp16split", "fp16pack")
    packed = mode == "fp16pack"
    mm_np = np.float16 if split else np.float32

    def pack_rows(hi, lo, ncol):
        # [xh; xh; xl] (x side) or [wh; wl; wh] (w side) + zero pad to 19*128
        flat = np.zeros((KR * 128, ncol), np.float16)
        cat = np.concatenate(hi + lo, axis=0)       # [2352, ncol]
        flat[:cat.shape[0]] = cat
        return np.ascontiguousarray(
            flat.reshape(KR, 128, ncol).transpose(1, 0, 2))

    def hilo(a):
        hi = a.astype(np.float16)
        lo = (a - hi.astype(np.float32)).astype(np.float16)
        return hi, lo

    data = np.asarray(data, dtype=np.float32)
    xT = np.ascontiguousarray(data.transpose(2, 0, 1))  # [784, T, 512]
    w1T = np.ascontiguousarray(
        np.asarray(w1, np.float32).T.reshape(KT, KP, NHID).transpose(1, 0, 2))
    w2T = np.ascontiguousarray(
        np.asarray(w2, np.float32).T.reshape(MT, 128, NOUT).transpose(1, 0, 2))
    b1_l = np.ascontiguousarray(
        np.asarray(b1, np.float32).reshape(MT, 128).T)
    b2_l = np.asarray(b2, np.float32).reshape(NOUT, 1).copy()

    shared = {"b1_in": b1_l, "b2_in": b2_l}
    if split:
        w1T_flat = np.ascontiguousarray(
            w1T.transpose(1, 0, 2)).reshape(NIN, NHID)   # rows in k order
        w1h, w1l = hilo(w1T_flat)
        if packed:
            # corrections first, hi*hi last: psum rounding stays tiny while
            # the small terms accumulate
            shared["w1_wh"] = pack_rows([w1l, w1h], [w1h], NHID)
        else:
            shared["w1_wh"] = np.ascontiguousarray(
                w1h.reshape(KT, KP, NHID).transpose(1, 0, 2))
            shared["w1_wl"] = np.ascontiguousarray(
                w1l.reshape(KT, KP, NHID).transpose(1, 0, 2))
        w2h, w2l = hilo(w2T)
        w2cat = np.zeros((128, MT, 32 + NOUT), np.float16)
        w2cat[:, :, :NOUT] = w2h
        w2cat[:, :, 32:32 + NOUT] = w2l
        shared["w2_wh"] = w2cat
    else:
        shared["w1_wh"] = w1T.astype(mm_np)
        shared["w2_wh"] = w2T.astype(mm_np)

    in_maps = []
    for ci in range(NCORES):
        xc = np.ascontiguousarray(
            xT[:, :, ci * BL:(ci + 1) * BL])           # [784, T, 64]
        xc = np.ascontiguousarray(
            xc.reshape(KT, KP, t_total * BL).transpose(1, 0, 2))
        m = dict(shared)
        if split and packed:
            xc_flat = np.ascontiguousarray(
                xc.transpose(1, 0, 2)).reshape(NIN, t_total * BL)
            xh, xl = hilo(xc_flat)
            m["x_xh"] = pack_rows([xh, xl], [xh], t_total * BL)
        elif split:
            m["x_xh"], m["x_xl"] = hilo(xc)
        else:
            m["x_xh"] = xc.astype(mm_np)
        in_maps.append(m)
    return in_maps


_NC_CACHE = {}


MODE = "fp16pack"
TRACE = False          # test-harness hook: capture NTFF profile on next run
LAST_RESULTS = None    # BassKernelResults of the last traced run


def kernel(data, w1, b1, w2, b2, num_steps):
    t_real = int(num_steps)
    data = np.asarray(data, dtype=np.float32)
    assert t_real == data.shape[0]
    t_total = ((t_real + TC - 1) // TC) * TC
    if t_total != t_real:
        pad = np.zeros((t_total - t_real,) + data.shape[1:], np.float32)
        data = np.concatenate([data, pad], axis=0)
    key = (t_total, MODE)
    if key not in _NC_CACHE:
        _NC_CACHE[key] = build_nc(t_total, MODE)
    nc = _NC_CACHE[key]
    in_maps = _prep_inputs(data, w1, b1, w2, b2, t_total, MODE)
    if TRACE:
        global LAST_RESULTS
        res = bass_utils.run_bass_kernel_spmd(
            nc, in_maps, core_ids=list(range(NCORES)), trace=True)
        LAST_RESULTS = res
    else:
        res = bass_utils.run_bass_kernel_spmd(
            nc, in_maps, core_ids=list(range(NCORES)))
    out = np.empty((t_total, B, NOUT), dtype=np.float32)
    for ci in range(NCORES):
        o = res.results[ci]["o_out"].reshape(NOUT, t_total, BL)
        out[:, ci * BL:(ci + 1) * BL, :] = o.transpose(1, 2, 0)
    return np.ascontiguousarray(out[:t_real])

